# revision 27
# baseline (speedup 1.0000x reference)
"""Trainium2 Bass kernel: out = 2 * cummax_W(cummax_H(x)) for x [16,256,128,128] f32.

v6: pair-trick on BOTH passes; the W-pass odd-result merge (into the PE
transpose input tile) runs on the SCALAR engine (avoids SWDGE descriptor-ring
SBUF traffic that taxed the DVE when the merge was a gpsimd DMA).

See kernel_v4_hpair.py docstring for the pair-trick layout details.
"""

from contextlib import ExitStack

import numpy as np

import concourse.bass as bass
import concourse.tile as tile
from concourse import bacc, mybir
from concourse.bass_utils import run_bass_kernel_spmd
from concourse.masks import make_identity

N_CORES = 8
B, C, H, W = 16, 256, 128, 128
S = (B // N_CORES) * C
NEG = -3.0e38

F32 = mybir.dt.float32
BF16 = mybir.dt.bfloat16

LAST_RESULTS = None


def build_nc(n_slices: int = S, g: int = 16, bufs: int = 6, taper: int = 2) -> bass.Bass:
    nc = bacc.Bacc(None, target_bir_lowering=False)
    # h-major input with W deinterleaved: x[h, s*W + (wE|wO)]
    # output o[w', s*128 + (hE|hO)], w' = (evens | odds)
    x = nc.declare_dram_parameter("x", [H, n_slices * W], BF16, isOutput=False)
    cst = nc.declare_dram_parameter("cst", [128, 128 + g * 66], BF16, isOutput=False)
    o = nc.declare_dram_parameter("o", [W, n_slices * H], BF16, isOutput=True)

    head = [4, 4, 8]
    tail_t = [8, 4, 4]
    chunks = []
    pos = 0
    for c in head:
        chunks.append((pos, c))
        pos += c
    tail = n_slices - sum(tail_t)
    while pos < tail:
        chunks.append((pos, g))
        pos += g
    for c in tail_t:
        chunks.append((pos, c))
        pos += c
    assert pos == n_slices, (pos, n_slices)

    with ExitStack() as ctx:
        tc = ctx.enter_context(tile.TileContext(nc))
        consts = ctx.enter_context(tc.tile_pool(name="consts", bufs=1))
        cstt = consts.tile([128, 128 + g * 66], BF16)
        nc.sync.dma_start(out=cstt[:], in_=cst.ap())
        ident = cstt[:, 0:128]
        bias_m = cstt[:, 128 : 128 + g * 66]

        xpool = ctx.enter_context(tc.tile_pool(name="xt", bufs=bufs))
        apool = ctx.enter_context(tc.tile_pool(name="at", bufs=bufs))
        epool = ctx.enter_context(tc.tile_pool(name="be", bufs=bufs))
        opool = ctx.enter_context(tc.tile_pool(name="bo", bufs=bufs))
        mpool = ctx.enter_context(tc.tile_pool(name="mt", bufs=2))
        zwpool = ctx.enter_context(tc.tile_pool(name="zw", bufs=bufs))
        zhpool = ctx.enter_context(tc.tile_pool(name="zh", bufs=bufs))
        rpool = ctx.enter_context(tc.tile_pool(name="rt", bufs=bufs))
        pa_pool = ctx.enter_context(tc.tile_pool(name="pa", bufs=6, space="PSUM"))

        xv = x.ap()
        ov = o.ap()

        for ci, (s0, gc) in enumerate(chunks):
            fw = gc * W
            hw = gc * 64
            xt = xpool.tile([128, fw], BF16, tag="xt")
            nc.sync.dma_start(out=xt[:], in_=xv[:, s0 * W : s0 * W + fw])
            xts = xt[:].rearrange("p (s e) -> p s e", s=gc)

            # --- W pass (pair trick) ---
            mtw = mpool.tile([128, g * 66], BF16, tag="mtw")
            mtwv = mtw[:, : gc * 66].rearrange("p (s e) -> p s e", s=gc)
            if ci < 2:
                mf = mtw[:].rearrange("p (s e) -> p s e", s=g)
                nc.gpsimd.memset(mf[:, :, 0:2], NEG)
            nc.vector.tensor_tensor(
                mtwv[:, :, 2:66], xts[:, :, 0:64], xts[:, :, 64:128],
                mybir.AluOpType.max,
            )
            zw = zwpool.tile([128, gc * 66 + 4], BF16, tag="zw")
            nc.vector.tensor_tensor_scan(
                zw[:, 1 : gc * 66 + 1], bias_m[:, : gc * 66], mtw[:, : gc * 66],
                0.0, mybir.AluOpType.add, mybir.AluOpType.max,
            )
            zwv = zw[:, : gc * 66].rearrange("p (s e) -> p s e", s=gc)
            at = apool.tile([128, fw], BF16, tag="at")
            ats = at[:].rearrange("p (s e) -> p s e", s=gc)
            nc.vector.tensor_tensor(
                ats[:, :, 0:64], zwv[:, :, 2:66], xts[:, :, 0:64],
                mybir.AluOpType.max,
            )
            zwz = zw[:, 3 : 3 + gc * 66].rearrange("p (s e) -> p s e", s=gc)
            nc.scalar.copy(ats[:, :, 64:128], zwz[:, :, 0:64])

            # --- transpose + deinterleaved scalar staging ---
            btE = epool.tile([128, hw], BF16, tag="be")
            btO = opool.tile([128, hw], BF16, tag="bo")
            btEv = btE[:].rearrange("p (s e) -> p s e", s=gc)
            btOv = btO[:].rearrange("p (s e) -> p s e", s=gc)
            nb = max(1, gc // 8)
            sl = gc // nb
            for hb in range(nb):
                pa = pa_pool.tile([128, sl * 128], BF16, tag="pa")
                for j in range(sl):
                    s = hb * sl + j
                    nc.tensor.transpose(
                        pa[:, j * 128 : (j + 1) * 128],
                        at[:, s * 128 : (s + 1) * 128],
                        ident[:],
                    )
                pav = pa[:].rearrange("p (s hj hb) -> p s hj hb", s=sl, hb=2)
                nc.scalar.copy(btEv[:, hb * sl : (hb + 1) * sl], pav[:, :, :, 0])
                nc.scalar.copy(btOv[:, hb * sl : (hb + 1) * sl], pav[:, :, :, 1])

            # --- H pass (pair trick) ---
            mth = mpool.tile([128, g * 66], BF16, tag="mth")
            mthv = mth[:, : gc * 66].rearrange("p (s e) -> p s e", s=gc)
            if ci < 2:
                mf = mth[:].rearrange("p (s e) -> p s e", s=g)
                nc.gpsimd.memset(mf[:, :, 0:2], NEG)
            nc.vector.tensor_tensor(
                mthv[:, :, 2:66], btEv[:], btOv[:], mybir.AluOpType.max
            )
            zh = zhpool.tile([128, gc * 66 + 4], BF16, tag="zh")
            nc.vector.tensor_tensor_scan(
                zh[:, 1 : gc * 66 + 1], bias_m[:, : gc * 66], mth[:, : gc * 66],
                0.0, mybir.AluOpType.add, mybir.AluOpType.max,
            )
            zhv = zh[:, : gc * 66].rearrange("p (s e) -> p s e", s=gc)
            rt = rpool.tile([128, hw], BF16, tag="rt")
            rts = rt[:].rearrange("p (s e) -> p s e", s=gc)
            nc.vector.tensor_tensor(
                rts[:], zhv[:, :, 2:66], btEv[:], mybir.AluOpType.max
            )
            ovv = ov[:, s0 * H : s0 * H + fw].rearrange("p (s e) -> p s e", s=gc)
            nc.scalar.dma_start(out=ovv[:, :, 0:64], in_=rts[:])
            zhz = zh[:, 3 : 3 + gc * 66].rearrange("p (s e) -> p s e", s=gc)
            nc.gpsimd.dma_start(out=ovv[:, :, 64:128], in_=zhz[:, :, 0:64])
    nc.finalize()
    return nc


def kernel(x: np.ndarray) -> np.ndarray:
    global LAST_RESULTS
    import ml_dtypes

    assert x.shape == (B, C, H, W)
    xb = (np.asarray(x, dtype=np.float32) * 2.0).astype(ml_dtypes.bfloat16)
    xs = xb.reshape(N_CORES, S, H, W)
    xd = np.concatenate([xs[..., 0::2], xs[..., 1::2]], axis=-1)
    g = 16
    cst = np.zeros((128, 128 + g * 66), dtype=ml_dtypes.bfloat16)
    cst[:, 0:128] = np.eye(128, dtype=np.float32).astype(ml_dtypes.bfloat16)
    bias = np.zeros((128, g * 66), dtype=np.float32)
    bias[:, 0 : g * 66 : 66] = NEG
    cst[:, 128:] = bias.astype(ml_dtypes.bfloat16)
    in_maps = [
        {
            "x": np.ascontiguousarray(xd[i].transpose(1, 0, 2)).reshape(H, S * W),
            "cst": cst,
        }
        for i in range(N_CORES)
    ]
    nc = build_nc(S, g=16, bufs=6, taper=2)
    res = run_bass_kernel_spmd(nc, in_maps, core_ids=list(range(N_CORES)))
    LAST_RESULTS = res
    out = np.empty((N_CORES, S, H, W), dtype=np.float32)
    for i in range(N_CORES):
        oi = np.asarray(res.results[i]["o"]).reshape(W, S, 2, 64).astype(np.float32)
        t = np.empty((S, H, W), dtype=np.float32)
        t[:, 0::2, 0::2] = oi[0:64, :, 0, :].transpose(1, 2, 0)
        t[:, 1::2, 0::2] = oi[0:64, :, 1, :].transpose(1, 2, 0)
        t[:, 0::2, 1::2] = oi[64:128, :, 0, :].transpose(1, 2, 0)
        t[:, 1::2, 1::2] = oi[64:128, :, 1, :].transpose(1, 2, 0)
        out[i] = t
    return out.reshape(B, C, H, W)


# revision 28
# speedup vs baseline: 1.0100x; 1.0100x over previous
"""Trainium2 Bass kernel: out = 2 * cummax_W(cummax_H(x)) for x [16,256,128,128] f32.

v6: pair-trick on BOTH passes; the W-pass odd-result merge (into the PE
transpose input tile) runs on the SCALAR engine (avoids SWDGE descriptor-ring
SBUF traffic that taxed the DVE when the merge was a gpsimd DMA).

See kernel_v4_hpair.py docstring for the pair-trick layout details.
"""

from contextlib import ExitStack

import numpy as np

import concourse.bass as bass
import concourse.tile as tile
from concourse import bacc, mybir
from concourse.bass_utils import run_bass_kernel_spmd
from concourse.masks import make_identity

N_CORES = 8
B, C, H, W = 16, 256, 128, 128
S = (B // N_CORES) * C
NEG = -3.0e38

F32 = mybir.dt.float32
BF16 = mybir.dt.bfloat16

LAST_RESULTS = None


def build_nc(n_slices: int = S, g: int = 16, bufs: int = 6, taper: int = 2) -> bass.Bass:
    nc = bacc.Bacc(None, target_bir_lowering=False)
    # h-major input with W deinterleaved: x[h, s*W + (wE|wO)]
    # output o[w', s*128 + (hE|hO)], w' = (evens | odds)
    x = nc.declare_dram_parameter("x", [H, n_slices * W], BF16, isOutput=False)
    cst = nc.declare_dram_parameter("cst", [128, 128 + g * 66], BF16, isOutput=False)
    o = nc.declare_dram_parameter("o", [W, n_slices * H], BF16, isOutput=True)

    head = [8, 8]
    tail_t = [8, 8]
    chunks = []
    pos = 0
    for c in head:
        chunks.append((pos, c))
        pos += c
    tail = n_slices - sum(tail_t)
    while pos < tail:
        chunks.append((pos, g))
        pos += g
    for c in tail_t:
        chunks.append((pos, c))
        pos += c
    assert pos == n_slices, (pos, n_slices)

    with ExitStack() as ctx:
        tc = ctx.enter_context(tile.TileContext(nc))
        consts = ctx.enter_context(tc.tile_pool(name="consts", bufs=1))
        cstt = consts.tile([128, 128 + g * 66], BF16)
        nc.sync.dma_start(out=cstt[:], in_=cst.ap())
        ident = cstt[:, 0:128]
        bias_m = cstt[:, 128 : 128 + g * 66]

        xpool = ctx.enter_context(tc.tile_pool(name="xt", bufs=bufs))
        apool = ctx.enter_context(tc.tile_pool(name="at", bufs=bufs))
        epool = ctx.enter_context(tc.tile_pool(name="be", bufs=bufs))
        opool = ctx.enter_context(tc.tile_pool(name="bo", bufs=bufs))
        mpool = ctx.enter_context(tc.tile_pool(name="mt", bufs=2))
        zwpool = ctx.enter_context(tc.tile_pool(name="zw", bufs=bufs))
        zhpool = ctx.enter_context(tc.tile_pool(name="zh", bufs=bufs))
        rpool = ctx.enter_context(tc.tile_pool(name="rt", bufs=bufs))
        pa_pool = ctx.enter_context(tc.tile_pool(name="pa", bufs=6, space="PSUM"))

        xv = x.ap()
        ov = o.ap()

        for ci, (s0, gc) in enumerate(chunks):
            fw = gc * W
            hw = gc * 64
            xt = xpool.tile([128, fw], BF16, tag="xt")
            nc.sync.dma_start(out=xt[:], in_=xv[:, s0 * W : s0 * W + fw])
            xts = xt[:].rearrange("p (s e) -> p s e", s=gc)

            # --- W pass (pair trick) ---
            mtw = mpool.tile([128, g * 66], BF16, tag="mtw")
            mtwv = mtw[:, : gc * 66].rearrange("p (s e) -> p s e", s=gc)
            if ci < 2:
                mf = mtw[:].rearrange("p (s e) -> p s e", s=g)
                nc.gpsimd.memset(mf[:, :, 0:2], NEG)
            nc.vector.tensor_tensor(
                mtwv[:, :, 2:66], xts[:, :, 0:64], xts[:, :, 64:128],
                mybir.AluOpType.max,
            )
            zw = zwpool.tile([128, gc * 66 + 4], BF16, tag="zw")
            nc.vector.tensor_tensor_scan(
                zw[:, 1 : gc * 66 + 1], bias_m[:, : gc * 66], mtw[:, : gc * 66],
                0.0, mybir.AluOpType.add, mybir.AluOpType.max,
            )
            zwv = zw[:, : gc * 66].rearrange("p (s e) -> p s e", s=gc)
            at = apool.tile([128, fw], BF16, tag="at")
            ats = at[:].rearrange("p (s e) -> p s e", s=gc)
            nc.vector.tensor_tensor(
                ats[:, :, 0:64], zwv[:, :, 2:66], xts[:, :, 0:64],
                mybir.AluOpType.max,
            )
            zwz = zw[:, 3 : 3 + gc * 66].rearrange("p (s e) -> p s e", s=gc)
            nc.scalar.copy(ats[:, :, 64:128], zwz[:, :, 0:64])

            # --- transpose + deinterleaved scalar staging ---
            btE = epool.tile([128, hw], BF16, tag="be")
            btO = opool.tile([128, hw], BF16, tag="bo")
            btEv = btE[:].rearrange("p (s e) -> p s e", s=gc)
            btOv = btO[:].rearrange("p (s e) -> p s e", s=gc)
            nb = max(1, gc // 8)
            sl = gc // nb
            for hb in range(nb):
                pa = pa_pool.tile([128, sl * 128], BF16, tag="pa")
                for j in range(sl):
                    s = hb * sl + j
                    nc.tensor.transpose(
                        pa[:, j * 128 : (j + 1) * 128],
                        at[:, s * 128 : (s + 1) * 128],
                        ident[:],
                    )
                pav = pa[:].rearrange("p (s hj hb) -> p s hj hb", s=sl, hb=2)
                nc.scalar.copy(btEv[:, hb * sl : (hb + 1) * sl], pav[:, :, :, 0])
                nc.scalar.copy(btOv[:, hb * sl : (hb + 1) * sl], pav[:, :, :, 1])

            # --- H pass (pair trick) ---
            mth = mpool.tile([128, g * 66], BF16, tag="mth")
            mthv = mth[:, : gc * 66].rearrange("p (s e) -> p s e", s=gc)
            if ci < 2:
                mf = mth[:].rearrange("p (s e) -> p s e", s=g)
                nc.gpsimd.memset(mf[:, :, 0:2], NEG)
            nc.vector.tensor_tensor(
                mthv[:, :, 2:66], btEv[:], btOv[:], mybir.AluOpType.max
            )
            zh = zhpool.tile([128, gc * 66 + 4], BF16, tag="zh")
            nc.vector.tensor_tensor_scan(
                zh[:, 1 : gc * 66 + 1], bias_m[:, : gc * 66], mth[:, : gc * 66],
                0.0, mybir.AluOpType.add, mybir.AluOpType.max,
            )
            zhv = zh[:, : gc * 66].rearrange("p (s e) -> p s e", s=gc)
            rt = rpool.tile([128, hw], BF16, tag="rt")
            rts = rt[:].rearrange("p (s e) -> p s e", s=gc)
            nc.vector.tensor_tensor(
                rts[:], zhv[:, :, 2:66], btEv[:], mybir.AluOpType.max
            )
            ovv = ov[:, s0 * H : s0 * H + fw].rearrange("p (s e) -> p s e", s=gc)
            nc.scalar.dma_start(out=ovv[:, :, 0:64], in_=rts[:])
            zhz = zh[:, 3 : 3 + gc * 66].rearrange("p (s e) -> p s e", s=gc)
            nc.gpsimd.dma_start(out=ovv[:, :, 64:128], in_=zhz[:, :, 0:64])
    nc.finalize()
    return nc


def kernel(x: np.ndarray) -> np.ndarray:
    global LAST_RESULTS
    import ml_dtypes

    assert x.shape == (B, C, H, W)
    xb = (np.asarray(x, dtype=np.float32) * 2.0).astype(ml_dtypes.bfloat16)
    xs = xb.reshape(N_CORES, S, H, W)
    xd = np.concatenate([xs[..., 0::2], xs[..., 1::2]], axis=-1)
    g = 16
    cst = np.zeros((128, 128 + g * 66), dtype=ml_dtypes.bfloat16)
    cst[:, 0:128] = np.eye(128, dtype=np.float32).astype(ml_dtypes.bfloat16)
    bias = np.zeros((128, g * 66), dtype=np.float32)
    bias[:, 0 : g * 66 : 66] = NEG
    cst[:, 128:] = bias.astype(ml_dtypes.bfloat16)
    in_maps = [
        {
            "x": np.ascontiguousarray(xd[i].transpose(1, 0, 2)).reshape(H, S * W),
            "cst": cst,
        }
        for i in range(N_CORES)
    ]
    nc = build_nc(S, g=16, bufs=6, taper=2)
    res = run_bass_kernel_spmd(nc, in_maps, core_ids=list(range(N_CORES)))
    LAST_RESULTS = res
    out = np.empty((N_CORES, S, H, W), dtype=np.float32)
    for i in range(N_CORES):
        oi = np.asarray(res.results[i]["o"]).reshape(W, S, 2, 64).astype(np.float32)
        t = np.empty((S, H, W), dtype=np.float32)
        t[:, 0::2, 0::2] = oi[0:64, :, 0, :].transpose(1, 2, 0)
        t[:, 1::2, 0::2] = oi[0:64, :, 1, :].transpose(1, 2, 0)
        t[:, 0::2, 1::2] = oi[64:128, :, 0, :].transpose(1, 2, 0)
        t[:, 1::2, 1::2] = oi[64:128, :, 1, :].transpose(1, 2, 0)
        out[i] = t
    return out.reshape(B, C, H, W)
